# revision 57
# baseline (speedup 1.0000x reference)
"""Trainium2 Bass kernel for nn_FIB_RNN (GRU encoder + autoregressive
sampling decoder with DenseVariational head).

Contract: kernel(**inputs) takes the FULL unsharded inputs (numpy arrays,
keys as in reference.setup_inputs()) and returns the FULL output
[B, GAMMA, 2] float32.

Strategy: pure data parallelism over the batch dim across 8 NeuronCores
(1024 batch rows per core), feature-major GRU state [U=128, batch],
2 batch chunks of 512 pipelined against each other.  The per-chunk
recurrent chain (matmul -> sigmoid -> mul/add -> tanh -> h-update) is
the binding constraint, so the design optimizes chain latency
(873us baseline -> ~685us):

 - fp16 matmul operands (2x stream rate + FWL fast weight loads; the
   10-bit mantissa keeps the recurrent rounding error ~8x below bf16,
   final rel err 1.5e-3 vs the 2e-2 gate).
 - gates reordered {r, -z, h} with z negated: r and u1=1-z come from
   plain sigmoid; the r-half activation is emitted separately so the
   reset-mul can start before the z-half is done.
 - gate input biases ride a ones-row in an augmented K=2 encoder input
   matmul; h-gate bias via the tanh ACT bias (all biases are zero for
   this problem's inputs anyway).
 - decoder head/sample pipeline runs on [33, 512] tiles with chunk 1's
   row at partition 32 via matmul tile_position, so Exp, Ln(x+1),
   AFFINE_MUL_REDUCE and the y-assembly scalar_tensor_tensor all run at
   512-element latency instead of 1024 on a single lane.  (Non-matmul
   engine ops must still start at partition 0 - base-32 ACT/DVE APs
   break on hardware - so those ops cover the whole [33, 512] tile.)
 - softplus via Exp then Ln(x+1), pinned to the
   natural_log_exp_and_others table set (one table-load pair per
   decoder step, loads overlapped with the head matmuls).
 - outputs: raw exp row E and the fp16 feedback sample y are DMA'd;
   the host computes scale = 1e-5+0.05*log1p(E) and reconstructs
   loc = y - scale*eps exactly.
 - chain emission is phase-ordered across chunks (all sigmoids, then
   tt/uu, then tanh, then h-updates) so the in-order ACT/DVE queues
   never head-of-line block a ready op behind the other chunk's stall.
 - dependency-free warming matmuls in BOTH loops fill PE idle gaps to
   keep the HAM activity window busy (cold 1.2 GHz vs warm 2.4 GHz PE;
   measured ~600ns vs ~370ns per [128,128]@[128,512] matmul).  Removing
   the decoder's warming costs >100us - the warmth carries into the
   next step's matmul burst.
 - PSUM plan (8 banks): psrz [128,1024] bufs=2 (4) + psh [128,512]
   bufs=2 (2) + psx/junk [128,512] bufs=1 (1) + phead [33,512] bufs=1
   (1).
"""

import os
import sys
from contextlib import ExitStack

import numpy as np

for _p in ("/opt/trn_rl_repo", "/root/.axon_site/_ro/trn_rl_repo"):
    if os.path.isdir(_p) and _p not in sys.path:
        sys.path.insert(0, _p)

import concourse.bass as bass
import concourse.tile as tile
from concourse import bacc, mybir
from concourse.bass_utils import run_bass_kernel_spmd

F32 = mybir.dt.float32
AF = mybir.ActivationFunctionType
ALU = mybir.AluOpType
_MM_MODE = os.environ.get("KERNEL_MM_DT", "fp16")
RD = {"fp16": mybir.dt.float16, "f32r": mybir.dt.float32r}[_MM_MODE]
RD16 = mybir.dt.float16 if _MM_MODE == "fp16" else F32

U = 128                    # rnn units
T_ENC = 48                 # encoder steps
GAMMA = 28                 # decoder outputs (27 sampled feedback steps)
N_CORES = 8
B_FULL = 8192
BC = B_FULL // N_CORES     # 1024 batch rows per core
CW = 512                   # chunk width (PSUM bank = 512 fp32)
NCH = BC // CW             # 2 chunks per core
PH = 33                    # head tile partition span ({0, 32} rows used)
C_SP = float(np.log(np.expm1(1.0)))  # softplus^-1(1.0)
Q_SCALE = 0.02
OP_SCALE = 0.05

_CACHE = {}


def _round_rd(a):
    """Cast fp32 array to the matmul operand dtype's numpy storage."""
    a = np.ascontiguousarray(a, np.float32)
    if _MM_MODE == "fp16":
        return np.ascontiguousarray(a.astype(np.float16))
    bits = a.view(np.uint32)
    out = ((bits.astype(np.uint64) + 0x800) & 0xFFFFF000).astype(np.uint32)
    return out.view(np.float32)


def _pin_act_tables(arch):
    """Hide Exp/Ln from the single-function table sets so the compiler's
    table-load placement resolves both to natural_log_exp_and_others
    (one load covers the decoder's Exp+Ln pair).  Mutates the cached
    dict in place; set positions (= walrus set ids) are unchanged, and
    the real on-device tables still contain the hidden entries, so this
    only steers placement, never correctness."""
    from concourse.hw_specs import get_activation_tables

    tabs = get_activation_tables(arch)
    for name in ("exp_and_others", "exp_and_friends"):
        if name in tabs:
            tabs[name].discard(AF.Exp)
    if "natural_log" in tabs:
        tabs["natural_log"].discard(AF.Ln)


def _build_program(with_b1h):
    """Build + schedule the single-core Bass program (shared by all 8
    cores; per-core data differs only through the input tensors)."""
    nc = bacc.Bacc("TRN2", target_bir_lowering=False, debug=False)
    _pin_act_tables(nc.m.arch)

    # DRAM tensors.  Gate order everywhere is {r, -z, h}.
    x2_seq = nc.dram_tensor("x2_seq", [T_ENC, 2, BC], RD, kind="ExternalInput").ap()
    xb_seq = nc.dram_tensor("xb_seq", [T_ENC, BC], RD, kind="ExternalInput").ap()
    eps_seq = nc.dram_tensor("eps_seq", [GAMMA - 1, NCH, CW], F32, kind="ExternalInput").ap()
    r_w = nc.dram_tensor("r_w", [U, 3 * U], RD, kind="ExternalInput").ap()
    k2_w = nc.dram_tensor("k2_w", [2, 3 * U], RD, kind="ExternalInput").ap()
    k33_w = nc.dram_tensor("k33_w", [PH, 3 * U], RD, kind="ExternalInput").ap()
    wk = nc.dram_tensor("wk", [U, 2 * GAMMA], RD, kind="ExternalInput").ap()
    wk33 = nc.dram_tensor("wk33", [U, 2 * PH * GAMMA], RD, kind="ExternalInput").ap()
    wb33 = nc.dram_tensor("wb33", [PH, GAMMA], F32, kind="ExternalInput").ap()
    cvb33 = nc.dram_tensor("cvb33", [PH, GAMMA], F32, kind="ExternalInput").ap()
    gb = nc.dram_tensor("gb", [U, 2], F32, kind="ExternalInput").ap()
    k_col = nc.dram_tensor("k_col", [U, 1], F32, kind="ExternalInput").ap()
    h0_z = nc.dram_tensor("h0_z", [U, BC], RD, kind="ExternalInput").ap()
    out_E = nc.dram_tensor("out_E", [GAMMA, NCH, CW], F32, kind="ExternalOutput").ap()
    out_y = nc.dram_tensor("out_y", [GAMMA - 1, NCH, CW], RD, kind="ExternalOutput").ap()
    out_l27 = nc.dram_tensor("out_l27", [NCH, CW], F32, kind="ExternalOutput").ap()

    with tile.TileContext(nc) as tc, ExitStack() as es:
        consts = es.enter_context(tc.tile_pool(name="consts", bufs=1))
        R = consts.tile([U, 3 * U], RD)
        K2 = consts.tile([2, 3 * U], RD)
        K33 = consts.tile([PH, 3 * U], RD)
        WK = consts.tile([U, 2 * GAMMA], RD)
        WK33 = consts.tile([U, 2 * PH * GAMMA], RD)
        WB = consts.tile([PH, GAMMA], F32)
        CVB = consts.tile([PH, GAMMA], F32)
        GB = consts.tile([U, 2], F32)
        KC = consts.tile([U, 1], F32)
        EP = consts.tile([PH, CW], F32, name="ep")
        ACC = consts.tile([PH, 1], F32)
        nc.sync.dma_start(R[:], r_w[:])
        nc.sync.dma_start(K2[:], k2_w[:])
        nc.sync.dma_start(K33[:], k33_w[:])
        nc.sync.dma_start(WK[:], wk[:])
        nc.sync.dma_start(WK33[:], wk33[:])
        nc.sync.dma_start(WB[:], wb33[:])
        nc.sync.dma_start(CVB[:], cvb33[:])
        nc.sync.dma_start(GB[:], gb[:])
        nc.sync.dma_start(KC[:], k_col[:])
        nc.vector.memset(EP[:], 0.0)

        hpool = es.enter_context(tc.tile_pool(name="h", bufs=3))
        gates = es.enter_context(tc.tile_pool(name="gates", bufs=2))
        samp = es.enter_context(tc.tile_pool(name="samp", bufs=2))
        stage = es.enter_context(tc.tile_pool(name="stage", bufs=3))
        ps_g = es.enter_context(tc.tile_pool(name="psg", bufs=2, space="PSUM"))

        h = []
        for c in range(NCH):
            hc = hpool.tile([U, CW], RD, tag=f"h{c}")
            nc.sync.dma_start(hc[:], h0_z[:, bass.ts(c, CW)])
            h.append(hc)

        rs = bass.ts(0, U)       # gate column ranges in R/K: {r, -z, h}
        zs = bass.ts(1, U)
        hs = bass.ts(2, U)

        def gru_mm(c, x2=None, Y=None):
            """Gate matmuls for chunk c.  Encoder: x2 [2,BC] (row1=ones
            for biases); h-gate x-term comes via the xb STT instead of a
            matmul.  Decoder: Y [33,CW] fp16 (row 32c = y_c).
            Returns (psrz, psh, psx)."""
            hc = h[c]
            psrz = ps_g.tile([U, 2 * CW], F32, tag="psrz", bufs=2)
            psh = ps_g.tile([U, CW], F32, tag="psh", bufs=2)
            psx = None
            if Y is None:
                cs = bass.ts(c, CW)
                nc.tensor.matmul(psrz[:, 0:CW], K2[:, rs], x2[:, cs],
                                 start=True, stop=False)
                nc.tensor.matmul(psrz[:, 0:CW], R[:, rs], hc[:],
                                 start=False, stop=True)
                nc.tensor.matmul(psrz[:, CW:], K2[:, zs], x2[:, cs],
                                 start=True, stop=False)
                nc.tensor.matmul(psrz[:, CW:], R[:, zs], hc[:],
                                 start=False, stop=True)
                nc.tensor.matmul(psh[:], R[:, hs], hc[:], start=True, stop=True)
            else:
                p = 32 * c
                yr = Y[p : p + 1, :]
                kr = K33[p : p + 1, :]
                tp = (p, 0)
                psx = ps_g.tile([U, CW], F32, tag="psx", bufs=1)
                # R@h first (ready early), then K@y (y arrives late)
                nc.tensor.matmul(psrz[:, 0:CW], R[:, rs], hc[:],
                                 start=True, stop=False)
                nc.tensor.matmul(psrz[:, CW:], R[:, zs], hc[:],
                                 start=True, stop=False)
                nc.tensor.matmul(psh[:], R[:, hs], hc[:], start=True, stop=True)
                nc.tensor.matmul(psrz[:, 0:CW], kr[:, rs], yr,
                                 start=False, stop=True, tile_position=tp)
                nc.tensor.matmul(psrz[:, CW:], kr[:, zs], yr,
                                 start=False, stop=True, tile_position=tp)
                nc.tensor.matmul(psx[:], kr[:, hs], yr,
                                 start=True, stop=True, tile_position=tp)
            return psrz, psh, psx

        def gru_sig(c, psrz):
            """Gate sigmoids for chunk c (emitted for both chunks before
            any tanh so the in-order ACT queue never head-of-line blocks
            a ready sigmoid behind the other chunk's tanh)."""
            G = gates.tile([U, 2 * CW], RD16, tag=f"G_{c}")
            # r-half first: the reset-mul only needs this half
            nc.scalar.activation(G[:, 0:CW], psrz[:, 0:CW], AF.Sigmoid,
                                 bias=0.0, scale=1.0)
            nc.scalar.activation(G[:, CW:], psrz[:, CW:], AF.Sigmoid,
                                 bias=0.0, scale=1.0)
            return G

        def gru_mid(c, G, psh, psx, xb=None):
            """tt/uu for chunk c (both chunks emitted before any tanh)."""
            hrec = psh
            if with_b1h:
                hb = gates.tile([U, CW], F32, tag=f"hb_{c}")
                nc.vector.tensor_scalar(hb[:], psh[:], GB[:, 1:2], None,
                                        op0=ALU.add)
                hrec = hb
            tt = gates.tile([U, CW], F32, tag=f"t_{c}")
            nc.vector.tensor_mul(tt[:], G[:, 0:CW], hrec[:])
            uu = gates.tile([U, CW], F32, tag=f"u_{c}")
            if psx is None:
                nc.vector.scalar_tensor_tensor(
                    uu[:], xb[:, bass.ts(c, CW)], KC[:, 0:1], tt[:],
                    op0=ALU.mult, op1=ALU.add,
                )
            else:
                nc.vector.tensor_add(uu[:], tt[:], psx[:])
            return uu

        def gru_tanh(c, uu):
            hh = gates.tile([U, CW], RD16, tag=f"hh_{c}")
            nc.scalar.activation(hh[:], uu[:], AF.Tanh, bias=GB[:, 0:1],
                                 scale=1.0)
            return hh

        def gru_upd(c, G, hh):
            hc = h[c]
            d = gates.tile([U, CW], RD16, tag=f"d_{c}")
            nc.vector.tensor_sub(d[:], hh[:], hc[:])
            e = gates.tile([U, CW], RD16, tag=f"e_{c}")
            nc.vector.tensor_mul(e[:], G[:, CW:], d[:])
            h2 = hpool.tile([U, CW], RD, tag=f"h{c}")
            nc.vector.tensor_add(h2[:], hc[:], e[:])
            h[c] = h2

        def gru_chains(ps, xb=None):
            """Phase-ordered chains for both chunks."""
            G = [gru_sig(c, ps[c][0]) for c in range(NCH)]
            uu = [gru_mid(c, G[c], ps[c][1], ps[c][2], xb=xb)
                  for c in range(NCH)]
            hh = [gru_tanh(c, uu[c]) for c in range(NCH)]
            for c in range(NCH):
                gru_upd(c, G[c], hh[c])

        def dense_head(t):
            """DenseVariational head for step t on [33, CW] tiles
            (chunk c at partition 32c).  Returns (psl, SP)."""
            # chunk0's matmul is M=33 with a zero-padded lhsT so every
            # PSUM row is written (CoreSim rejects partial-init reads);
            # chunk1 overwrites row 32.  Same 512-col stream cost.
            psv = ps_g.tile([PH, CW], F32, tag="phead", bufs=1)
            v33 = bass.ts(2 * t + 1, PH)
            nc.tensor.matmul(psv[:], WK33[:, v33], h[0][:],
                             start=True, stop=True)
            nc.tensor.matmul(psv[32:33, :], WK[:, 2 * t + 1 : 2 * t + 2],
                             h[1][:], start=True, stop=True,
                             tile_position=(0, 32))
            E = samp.tile([PH, CW], F32, tag="E")
            nc.scalar.activation(E[:], psv[:], AF.Exp,
                                 bias=CVB[:, t : t + 1], scale=1.0)
            for c in range(NCH):
                nc.sync.dma_start(out_E[t, c, :],
                                  E[32 * c : 32 * c + 1, :])
            psl = ps_g.tile([PH, CW], F32, tag="phead", bufs=1)
            l33 = bass.ts(2 * t, PH)
            nc.tensor.matmul(psl[:], WK33[:, l33], h[0][:],
                             start=True, stop=True)
            nc.tensor.matmul(psl[32:33, :], WK[:, 2 * t : 2 * t + 1],
                             h[1][:], start=True, stop=True,
                             tile_position=(0, 32))
            if t == GAMMA - 1:
                l27 = samp.tile([PH, CW], F32, tag="l27")
                nc.vector.tensor_scalar(l27[:], psl[:], WB[:, t : t + 1],
                                        None, op0=ALU.add)
                for c in range(NCH):
                    nc.sync.dma_start(out_l27[c, :],
                                      l27[32 * c : 32 * c + 1, :])
                return None, None
            SP = samp.tile([PH, CW], F32, tag="SP")
            nc.scalar.activation(SP[:], E[:], AF.Ln, bias=1.0, scale=1.0)
            return psl, SP

        def sample(t, psl, SP):
            """Y rows {0,32} = (psl + wb0_t) + (0.05*sp + 1e-5)*eps_t."""
            for c in range(NCH):
                nc.sync.dma_start(EP[32 * c : 32 * c + 1, :],
                                  eps_seq[t, c, :])
            M = samp.tile([PH, CW], F32, tag="M")
            nc.vector.affine_mul_reduce(
                M[:], ACC[:], SP[:], EP[:], OP_SCALE, 1e-5,
            )
            Y = samp.tile([PH, CW], RD, tag="Y")
            nc.vector.scalar_tensor_tensor(
                Y[:], psl[:], WB[:, t : t + 1], M[:],
                op0=ALU.add, op1=ALU.add,
            )
            for c in range(NCH):
                nc.sync.dma_start(out_y[t, c, :], Y[32 * c : 32 * c + 1, :])
            return Y

        # ---- encoder: 48 GRU steps over the input sequence ----
        for t in range(T_ENC):
            x2 = stage.tile([2, BC], RD, tag="x2")
            nc.sync.dma_start(x2[:], x2_seq[t, :, :])
            xb = stage.tile([U, BC], RD, tag="xb")
            nc.sync.dma_start(xb[:], xb_seq[t : t + 1, :].partition_broadcast(U))
            ps = [gru_mm(c, x2=x2) for c in range(NCH)]
            # dependency-free warming matmuls into a spare bank: they run
            # while the PE would otherwise stall on h2, keeping the HAM
            # activity window busy so real matmuls stream at 2.4 GHz.
            junk = ps_g.tile([U, CW], F32, tag="psx", bufs=1)
            for j in range(6):
                nc.tensor.matmul(junk[:], R[:, rs if j % 2 else zs],
                                 xb[:, 0:CW], start=(j == 0),
                                 stop=(j == 5))
            gru_chains(ps, xb=xb)

        # ---- decoder: dense head + 27 sampled feedback GRU steps ----
        psl, SP = dense_head(0)
        for t in range(1, GAMMA):
            Y = sample(t - 1, psl, SP)
            ps = [gru_mm(c, Y=Y) for c in range(NCH)]
            # two warming batches timed by their ring slots' WAR frees:
            # phead frees at the sample STT (fills the post-K@y hole),
            # psx frees at uu_c1 (fills the tanh/h-update phase)
            junkP = ps_g.tile([PH, CW], F32, tag="phead", bufs=1)
            for j in range(4):
                nc.tensor.matmul(junkP[:], WK33[:, 0:PH], h[0][:],
                                 start=(j == 0), stop=(j == 3))
            junk = ps_g.tile([U, CW], F32, tag="psx", bufs=1)
            for j in range(4):
                nc.tensor.matmul(junk[:], R[:, rs if j % 2 else zs],
                                 h[0][:], start=(j == 0), stop=(j == 3))
            gru_chains(ps)
            psl, SP = dense_head(t)

    nc.compile()
    return nc


def _host_prep(inputs, gru_kernel, gru_rec_kernel, gru_bias, dv_loc, dv_rho,
               dv_eps, samp_eps):
    """Host-side input preprocessing -> per-core input maps."""
    inputs = np.asarray(inputs, np.float32)
    B = inputs.shape[0]
    assert B == B_FULL, f"kernel compiled for B={B_FULL}, got {B}"
    xT = _round_rd(inputs[:, :T_ENC, 0].T).astype(np.float32)  # [48, B]
    epsT = np.ascontiguousarray(np.asarray(samp_eps, np.float32)[:, :, 0])  # [27, B]

    gru_bias = np.asarray(gru_bias, np.float32)
    b0, b1 = gru_bias[0], gru_bias[1]
    gk = np.asarray(gru_kernel, np.float32)[0]                 # [3U]
    rk = np.asarray(gru_rec_kernel, np.float32)                # [U, 3U]

    # gate reorder {z,r,h} -> {r, -z, h}; z columns negated
    def reorder(m, axis):
        z, r, hh_ = np.split(m, 3, axis=axis)
        return np.concatenate([r, -z, hh_], axis=axis)

    r_w = reorder(rk, 1)                                       # [U, 3U]
    k_row = reorder(gk[None, :], 1)                            # [1, 3U]
    bias_rz = reorder((b0 + b1)[None, :], 1)                   # [1, 3U]
    bias_rz[0, 2 * U :] = 0.0                                  # h-gate bias via ACT
    k2 = np.concatenate([k_row, bias_rz], axis=0)              # [2, 3U]
    # decoder assumes zero r/z input biases (true for this problem);
    # nonzero ones would need the ones-row path in the decoder too.
    assert not np.any(bias_rz[0, : 2 * U]), "nonzero gate biases unsupported"
    k33 = np.zeros((PH, 3 * U), np.float32)
    k33[0] = k_row[0]
    k33[32] = k_row[0]

    gb = np.zeros((U, 2), np.float32)
    gb[:, 0] = b0[2 * U : 3 * U]                               # tanh bias
    gb[:, 1] = b1[2 * U : 3 * U]                               # recurrent h bias

    dv_loc = np.asarray(dv_loc, np.float32)
    dv_rho = np.asarray(dv_rho, np.float32)
    dv_eps = np.asarray(dv_eps, np.float32)
    scale_q = np.float32(1e-5) + np.float32(Q_SCALE) * np.logaddexp(
        np.float32(C_SP) + dv_rho, np.float32(0.0), dtype=np.float32
    )
    w_all = dv_loc[None, :] + scale_q[None, :] * dv_eps        # [28, 258]
    wk = np.ascontiguousarray(
        w_all[:, : 2 * U].reshape(GAMMA, U, 2).transpose(1, 0, 2).reshape(U, 2 * GAMMA)
    )
    # [U, (2t+ch)*33] view: col 0 of each 33-block = the weight, rest 0
    wk33 = np.zeros((U, 2 * GAMMA, PH), np.float32)
    wk33[:, :, 0] = wk
    wk33 = np.ascontiguousarray(wk33.reshape(U, 2 * PH * GAMMA))
    wb33 = np.broadcast_to(w_all[:, 2 * U][None, :], (PH, GAMMA)).copy()
    cvb33 = np.broadcast_to(
        (np.float32(C_SP) + w_all[:, 2 * U + 1])[None, :], (PH, GAMMA)
    ).copy()

    np_rd = np.float16 if _MM_MODE == "fp16" else np.float32
    x2_seq = np.empty((T_ENC, 2, B_FULL), np_rd)
    x2_seq[:, 0, :] = xT
    x2_seq[:, 1, :] = 1.0

    shared = {
        "r_w": _round_rd(r_w),
        "k2_w": _round_rd(k2),
        "k33_w": _round_rd(k33),
        "wk": _round_rd(wk),
        "wk33": _round_rd(wk33),
        "wb33": wb33.astype(np.float32),
        "cvb33": cvb33.astype(np.float32),
        "gb": gb,
        "k_col": np.ascontiguousarray(gk[2 * U :, None]),      # [U,1] K_h col
        "h0_z": np.zeros((U, BC), np_rd),
    }
    in_maps = []
    for c in range(N_CORES):
        sl = slice(c * BC, (c + 1) * BC)
        in_maps.append(
            dict(
                shared,
                x2_seq=np.ascontiguousarray(x2_seq[:, :, sl]),
                xb_seq=np.ascontiguousarray(xT[:, sl].astype(np_rd)),
                eps_seq=np.ascontiguousarray(
                    epsT[:, sl].reshape(GAMMA - 1, NCH, CW)),
            )
        )
    return in_maps, bool(np.any(gb[:, 1] != 0.0)), epsT


def _get_nc(with_b1h=False):
    key = ("nc", with_b1h)
    if key not in _CACHE:
        _CACHE[key] = _build_program(with_b1h)
    return _CACHE[key]


def _postprocess(res_list, epsT):
    """Assemble [B, GAMMA, 2] from per-core {out_E, out_y, out_l27}."""
    out = np.empty((B_FULL, GAMMA, 2), np.float32)
    for c in range(N_CORES):
        sl = slice(c * BC, (c + 1) * BC)
        E = np.asarray(res_list[c]["out_E"], np.float64).reshape(GAMMA, BC)
        y = np.asarray(res_list[c]["out_y"], np.float64).reshape(GAMMA - 1, BC)
        l27 = np.asarray(res_list[c]["out_l27"], np.float64).reshape(BC)
        scale = 1e-5 + OP_SCALE * np.log1p(E)                  # [28, BC]
        loc = np.empty((GAMMA, BC))
        loc[:-1] = y - scale[:-1] * epsT[:, sl]
        loc[-1] = l27
        out[sl, :, 0] = loc.T
        out[sl, :, 1] = scale.T
    return out


def run(inputs_dict, trace=False, trace_kwargs=None):
    in_maps, with_b1h, epsT = _host_prep(**inputs_dict)
    nc = _get_nc(with_b1h)
    res = run_bass_kernel_spmd(
        nc, in_maps, list(range(N_CORES)), trace=trace,
        **(trace_kwargs or {}),
    )
    _CACHE["last_results"] = res
    return _postprocess(res.results, epsT)


def kernel(**inputs):
    return run(inputs, trace=bool(os.environ.get("KERNEL_TRACE")))


# revision 58
# speedup vs baseline: 1.0041x; 1.0041x over previous
"""Trainium2 Bass kernel for nn_FIB_RNN (GRU encoder + autoregressive
sampling decoder with DenseVariational head).

Contract: kernel(**inputs) takes the FULL unsharded inputs (numpy arrays,
keys as in reference.setup_inputs()) and returns the FULL output
[B, GAMMA, 2] float32.

Strategy: pure data parallelism over the batch dim across 8 NeuronCores
(1024 batch rows per core), feature-major GRU state [U=128, batch],
2 batch chunks of 512 pipelined against each other.  The per-chunk
recurrent chain (matmul -> sigmoid -> mul/add -> tanh -> h-update) is
the binding constraint, so the design optimizes chain latency
(873us baseline -> ~685us):

 - fp16 matmul operands (2x stream rate + FWL fast weight loads; the
   10-bit mantissa keeps the recurrent rounding error ~8x below bf16,
   final rel err 1.5e-3 vs the 2e-2 gate).
 - gates reordered {r, -z, h} with z negated: r and u1=1-z come from
   plain sigmoid; the r-half activation is emitted separately so the
   reset-mul can start before the z-half is done.
 - gate input biases ride a ones-row in an augmented K=2 encoder input
   matmul; h-gate bias via the tanh ACT bias (all biases are zero for
   this problem's inputs anyway).
 - decoder head/sample pipeline runs on [33, 512] tiles with chunk 1's
   row at partition 32 via matmul tile_position, so Exp, Ln(x+1),
   AFFINE_MUL_REDUCE and the y-assembly scalar_tensor_tensor all run at
   512-element latency instead of 1024 on a single lane.  (Non-matmul
   engine ops must still start at partition 0 - base-32 ACT/DVE APs
   break on hardware - so those ops cover the whole [33, 512] tile.)
 - softplus via Exp then Ln(x+1), pinned to the
   natural_log_exp_and_others table set (one table-load pair per
   decoder step, loads overlapped with the head matmuls).
 - outputs: raw exp row E and the fp16 feedback sample y are DMA'd;
   the host computes scale = 1e-5+0.05*log1p(E) and reconstructs
   loc = y - scale*eps exactly.
 - chain emission is phase-ordered across chunks (all sigmoids, then
   tt/uu, then tanh, then h-updates) so the in-order ACT/DVE queues
   never head-of-line block a ready op behind the other chunk's stall.
 - dependency-free warming matmuls in BOTH loops fill PE idle gaps to
   keep the HAM activity window busy (cold 1.2 GHz vs warm 2.4 GHz PE;
   measured ~600ns vs ~370ns per [128,128]@[128,512] matmul).  Removing
   the decoder's warming costs >100us - the warmth carries into the
   next step's matmul burst.
 - PSUM plan (8 banks): psrz [128,1024] bufs=2 (4) + psh [128,512]
   bufs=2 (2) + psx/junk [128,512] bufs=1 (1) + phead [33,512] bufs=1
   (1).
"""

import os
import sys
from contextlib import ExitStack

import numpy as np

for _p in ("/opt/trn_rl_repo", "/root/.axon_site/_ro/trn_rl_repo"):
    if os.path.isdir(_p) and _p not in sys.path:
        sys.path.insert(0, _p)

import concourse.bass as bass
import concourse.tile as tile
from concourse import bacc, mybir
from concourse.bass_utils import run_bass_kernel_spmd

F32 = mybir.dt.float32
AF = mybir.ActivationFunctionType
ALU = mybir.AluOpType
_MM_MODE = os.environ.get("KERNEL_MM_DT", "fp16")
RD = {"fp16": mybir.dt.float16, "f32r": mybir.dt.float32r}[_MM_MODE]
RD16 = mybir.dt.float16 if _MM_MODE == "fp16" else F32

U = 128                    # rnn units
T_ENC = 48                 # encoder steps
GAMMA = 28                 # decoder outputs (27 sampled feedback steps)
N_CORES = 8
B_FULL = 8192
BC = B_FULL // N_CORES     # 1024 batch rows per core
CW = 512                   # chunk width (PSUM bank = 512 fp32)
NCH = BC // CW             # 2 chunks per core
PH = 33                    # head tile partition span ({0, 32} rows used)
C_SP = float(np.log(np.expm1(1.0)))  # softplus^-1(1.0)
Q_SCALE = 0.02
OP_SCALE = 0.05

_CACHE = {}


def _round_rd(a):
    """Cast fp32 array to the matmul operand dtype's numpy storage."""
    a = np.ascontiguousarray(a, np.float32)
    if _MM_MODE == "fp16":
        return np.ascontiguousarray(a.astype(np.float16))
    bits = a.view(np.uint32)
    out = ((bits.astype(np.uint64) + 0x800) & 0xFFFFF000).astype(np.uint32)
    return out.view(np.float32)


def _pin_act_tables(arch):
    """Hide Exp/Ln from the single-function table sets so the compiler's
    table-load placement resolves both to natural_log_exp_and_others
    (one load covers the decoder's Exp+Ln pair).  Mutates the cached
    dict in place; set positions (= walrus set ids) are unchanged, and
    the real on-device tables still contain the hidden entries, so this
    only steers placement, never correctness."""
    from concourse.hw_specs import get_activation_tables

    tabs = get_activation_tables(arch)
    for name in ("exp_and_others", "exp_and_friends"):
        if name in tabs:
            tabs[name].discard(AF.Exp)
    if "natural_log" in tabs:
        tabs["natural_log"].discard(AF.Ln)


def _build_program(with_b1h):
    """Build + schedule the single-core Bass program (shared by all 8
    cores; per-core data differs only through the input tensors)."""
    nc = bacc.Bacc("TRN2", target_bir_lowering=False, debug=False)
    _pin_act_tables(nc.m.arch)

    # DRAM tensors.  Gate order everywhere is {r, -z, h}.
    x2_seq = nc.dram_tensor("x2_seq", [T_ENC, 2, BC], RD, kind="ExternalInput").ap()
    xb_seq = nc.dram_tensor("xb_seq", [T_ENC, BC], RD, kind="ExternalInput").ap()
    eps_seq = nc.dram_tensor("eps_seq", [GAMMA - 1, NCH, CW], F32, kind="ExternalInput").ap()
    r_w = nc.dram_tensor("r_w", [U, 3 * U], RD, kind="ExternalInput").ap()
    k2_w = nc.dram_tensor("k2_w", [2, 3 * U], RD, kind="ExternalInput").ap()
    k33_w = nc.dram_tensor("k33_w", [PH, 3 * U], RD, kind="ExternalInput").ap()
    wk = nc.dram_tensor("wk", [U, 2 * GAMMA], RD, kind="ExternalInput").ap()
    wk33 = nc.dram_tensor("wk33", [U, 2 * PH * GAMMA], RD, kind="ExternalInput").ap()
    wb33 = nc.dram_tensor("wb33", [PH, GAMMA], F32, kind="ExternalInput").ap()
    cvb33 = nc.dram_tensor("cvb33", [PH, GAMMA], F32, kind="ExternalInput").ap()
    gb = nc.dram_tensor("gb", [U, 2], F32, kind="ExternalInput").ap()
    k_col = nc.dram_tensor("k_col", [U, 1], F32, kind="ExternalInput").ap()
    h0_z = nc.dram_tensor("h0_z", [U, BC], RD, kind="ExternalInput").ap()
    out_E = nc.dram_tensor("out_E", [GAMMA, NCH, CW], F32, kind="ExternalOutput").ap()
    out_y = nc.dram_tensor("out_y", [GAMMA - 1, NCH, CW], RD, kind="ExternalOutput").ap()
    out_l27 = nc.dram_tensor("out_l27", [NCH, CW], F32, kind="ExternalOutput").ap()

    with tile.TileContext(nc) as tc, ExitStack() as es:
        consts = es.enter_context(tc.tile_pool(name="consts", bufs=1))
        R = consts.tile([U, 3 * U], RD)
        K2 = consts.tile([2, 3 * U], RD)
        K33 = consts.tile([PH, 3 * U], RD)
        WK = consts.tile([U, 2 * GAMMA], RD)
        WK33 = consts.tile([U, 2 * PH * GAMMA], RD)
        WB = consts.tile([PH, GAMMA], F32)
        CVB = consts.tile([PH, GAMMA], F32)
        GB = consts.tile([U, 2], F32)
        KC = consts.tile([U, 1], F32)
        EP = consts.tile([PH, CW], F32, name="ep")
        ACC = consts.tile([PH, 1], F32)
        nc.sync.dma_start(R[:], r_w[:])
        nc.sync.dma_start(K2[:], k2_w[:])
        nc.sync.dma_start(K33[:], k33_w[:])
        nc.sync.dma_start(WK[:], wk[:])
        nc.sync.dma_start(WK33[:], wk33[:])
        nc.sync.dma_start(WB[:], wb33[:])
        nc.sync.dma_start(CVB[:], cvb33[:])
        nc.sync.dma_start(GB[:], gb[:])
        nc.sync.dma_start(KC[:], k_col[:])
        nc.vector.memset(EP[:], 0.0)

        hpool = es.enter_context(tc.tile_pool(name="h", bufs=3))
        gates = es.enter_context(tc.tile_pool(name="gates", bufs=2))
        samp = es.enter_context(tc.tile_pool(name="samp", bufs=2))
        stage = es.enter_context(tc.tile_pool(name="stage", bufs=3))
        ps_g = es.enter_context(tc.tile_pool(name="psg", bufs=2, space="PSUM"))

        h = []
        for c in range(NCH):
            hc = hpool.tile([U, CW], RD, tag=f"h{c}")
            nc.sync.dma_start(hc[:], h0_z[:, bass.ts(c, CW)])
            h.append(hc)

        rs = bass.ts(0, U)       # gate column ranges in R/K: {r, -z, h}
        zs = bass.ts(1, U)
        hs = bass.ts(2, U)

        def gru_mm(c, x2=None, Y=None):
            """Gate matmuls for chunk c.  Encoder: x2 [2,BC] (row1=ones
            for biases); h-gate x-term comes via the xb STT instead of a
            matmul.  Decoder: Y [33,CW] fp16 (row 32c = y_c).
            Returns (psrz, psh, psx)."""
            hc = h[c]
            psrz = ps_g.tile([U, 2 * CW], F32, tag="psrz", bufs=2)
            psh = ps_g.tile([U, CW], F32, tag="psh", bufs=2)
            psx = None
            if Y is None:
                cs = bass.ts(c, CW)
                nc.tensor.matmul(psrz[:, 0:CW], K2[:, rs], x2[:, cs],
                                 start=True, stop=False)
                nc.tensor.matmul(psrz[:, 0:CW], R[:, rs], hc[:],
                                 start=False, stop=True)
                nc.tensor.matmul(psrz[:, CW:], K2[:, zs], x2[:, cs],
                                 start=True, stop=False)
                nc.tensor.matmul(psrz[:, CW:], R[:, zs], hc[:],
                                 start=False, stop=True)
                nc.tensor.matmul(psh[:], R[:, hs], hc[:], start=True, stop=True)
            else:
                p = 32 * c
                yr = Y[p : p + 1, :]
                kr = K33[p : p + 1, :]
                tp = (p, 0)
                psx = ps_g.tile([U, CW], F32, tag="psx", bufs=1)
                # R@h first (ready early), then K@y (y arrives late)
                nc.tensor.matmul(psrz[:, 0:CW], R[:, rs], hc[:],
                                 start=True, stop=False)
                nc.tensor.matmul(psrz[:, CW:], R[:, zs], hc[:],
                                 start=True, stop=False)
                nc.tensor.matmul(psh[:], R[:, hs], hc[:], start=True, stop=True)
                nc.tensor.matmul(psrz[:, 0:CW], kr[:, rs], yr,
                                 start=False, stop=True, tile_position=tp)
                nc.tensor.matmul(psrz[:, CW:], kr[:, zs], yr,
                                 start=False, stop=True, tile_position=tp)
                nc.tensor.matmul(psx[:], kr[:, hs], yr,
                                 start=True, stop=True, tile_position=tp)
            return psrz, psh, psx

        def gru_sig(c, psrz):
            """Gate sigmoids for chunk c (emitted for both chunks before
            any tanh so the in-order ACT queue never head-of-line blocks
            a ready sigmoid behind the other chunk's tanh)."""
            G = gates.tile([U, 2 * CW], RD16, tag=f"G_{c}")
            # r-half first: the reset-mul only needs this half
            nc.scalar.activation(G[:, 0:CW], psrz[:, 0:CW], AF.Sigmoid,
                                 bias=0.0, scale=1.0)
            nc.scalar.activation(G[:, CW:], psrz[:, CW:], AF.Sigmoid,
                                 bias=0.0, scale=1.0)
            return G

        def gru_mid(c, G, psh, psx, xb=None):
            """tt/uu for chunk c (both chunks emitted before any tanh)."""
            hrec = psh
            if with_b1h:
                hb = gates.tile([U, CW], F32, tag=f"hb_{c}")
                nc.vector.tensor_scalar(hb[:], psh[:], GB[:, 1:2], None,
                                        op0=ALU.add)
                hrec = hb
            tt = gates.tile([U, CW], F32, tag=f"t_{c}")
            nc.vector.tensor_mul(tt[:], G[:, 0:CW], hrec[:])
            uu = gates.tile([U, CW], F32, tag=f"u_{c}")
            if psx is None:
                nc.vector.scalar_tensor_tensor(
                    uu[:], xb[:, bass.ts(c, CW)], KC[:, 0:1], tt[:],
                    op0=ALU.mult, op1=ALU.add,
                )
            else:
                nc.vector.tensor_add(uu[:], tt[:], psx[:])
            return uu

        def gru_tanh(c, uu):
            hh = gates.tile([U, CW], RD16, tag=f"hh_{c}")
            nc.scalar.activation(hh[:], uu[:], AF.Tanh, bias=GB[:, 0:1],
                                 scale=1.0)
            return hh

        def gru_upd(c, G, hh):
            hc = h[c]
            d = gates.tile([U, CW], RD16, tag=f"d_{c}")
            nc.vector.tensor_sub(d[:], hh[:], hc[:])
            e = gates.tile([U, CW], RD16, tag=f"e_{c}")
            nc.vector.tensor_mul(e[:], G[:, CW:], d[:])
            h2 = hpool.tile([U, CW], RD, tag=f"h{c}")
            nc.vector.tensor_add(h2[:], hc[:], e[:])
            h[c] = h2

        def gru_chains(ps, xb=None):
            """Phase-ordered chains for both chunks."""
            G = [gru_sig(c, ps[c][0]) for c in range(NCH)]
            uu = [gru_mid(c, G[c], ps[c][1], ps[c][2], xb=xb)
                  for c in range(NCH)]
            hh = [gru_tanh(c, uu[c]) for c in range(NCH)]
            for c in range(NCH):
                gru_upd(c, G[c], hh[c])

        def dense_head(t):
            """DenseVariational head for step t on [33, CW] tiles
            (chunk c at partition 32c).  Returns (psl, SP)."""
            # chunk0's matmul is M=33 with a zero-padded lhsT so every
            # PSUM row is written (CoreSim rejects partial-init reads);
            # chunk1 overwrites row 32.  Same 512-col stream cost.
            psv = ps_g.tile([PH, CW], F32, tag="phead", bufs=1)
            v33 = bass.ts(2 * t + 1, PH)
            nc.tensor.matmul(psv[:], WK33[:, v33], h[0][:],
                             start=True, stop=True)
            nc.tensor.matmul(psv[32:33, :], WK[:, 2 * t + 1 : 2 * t + 2],
                             h[1][:], start=True, stop=True,
                             tile_position=(0, 32))
            E = samp.tile([PH, CW], F32, tag="E")
            nc.scalar.activation(E[:], psv[:], AF.Exp,
                                 bias=CVB[:, t : t + 1], scale=1.0)
            for c in range(NCH):
                nc.sync.dma_start(out_E[t, c, :],
                                  E[32 * c : 32 * c + 1, :])
            psl = ps_g.tile([PH, CW], F32, tag="phead", bufs=1)
            l33 = bass.ts(2 * t, PH)
            nc.tensor.matmul(psl[:], WK33[:, l33], h[0][:],
                             start=True, stop=True)
            nc.tensor.matmul(psl[32:33, :], WK[:, 2 * t : 2 * t + 1],
                             h[1][:], start=True, stop=True,
                             tile_position=(0, 32))
            if t == GAMMA - 1:
                l27 = samp.tile([PH, CW], F32, tag="l27")
                nc.vector.tensor_scalar(l27[:], psl[:], WB[:, t : t + 1],
                                        None, op0=ALU.add)
                for c in range(NCH):
                    nc.sync.dma_start(out_l27[c, :],
                                      l27[32 * c : 32 * c + 1, :])
                return None, None
            SP = samp.tile([PH, CW], F32, tag="SP")
            nc.scalar.activation(SP[:], E[:], AF.Ln, bias=1.0, scale=1.0)
            return psl, SP

        def sample(t, psl, SP):
            """Y rows {0,32} = (psl + wb0_t) + (0.05*sp + 1e-5)*eps_t."""
            for c in range(NCH):
                nc.sync.dma_start(EP[32 * c : 32 * c + 1, :],
                                  eps_seq[t, c, :])
            M = samp.tile([PH, CW], F32, tag="M")
            nc.vector.affine_mul_reduce(
                M[:], ACC[:], SP[:], EP[:], OP_SCALE, 1e-5,
            )
            Y = samp.tile([PH, CW], RD, tag="Y")
            nc.vector.scalar_tensor_tensor(
                Y[:], psl[:], WB[:, t : t + 1], M[:],
                op0=ALU.add, op1=ALU.add,
            )
            for c in range(NCH):
                nc.sync.dma_start(out_y[t, c, :], Y[32 * c : 32 * c + 1, :])
            return Y

        # ---- encoder: 48 GRU steps over the input sequence ----
        for t in range(T_ENC):
            x2 = stage.tile([2, BC], RD, tag="x2")
            nc.sync.dma_start(x2[:], x2_seq[t, :, :])
            xb = stage.tile([U, BC], RD, tag="xb")
            nc.sync.dma_start(xb[:], xb_seq[t : t + 1, :].partition_broadcast(U))
            ps = [gru_mm(c, x2=x2) for c in range(NCH)]
            # dependency-free warming matmuls into a spare bank: they run
            # while the PE would otherwise stall on h2, keeping the HAM
            # activity window busy so real matmuls stream at 2.4 GHz.
            junk = ps_g.tile([U, CW], F32, tag="psx", bufs=1)
            for j in range(4):
                nc.tensor.matmul(junk[:], R[:, rs if j % 2 else zs],
                                 xb[:, 0:CW], start=(j == 0),
                                 stop=(j == 3))
            gru_chains(ps, xb=xb)

        # ---- decoder: dense head + 27 sampled feedback GRU steps ----
        psl, SP = dense_head(0)
        for t in range(1, GAMMA):
            Y = sample(t - 1, psl, SP)
            ps = [gru_mm(c, Y=Y) for c in range(NCH)]
            # warming matmuls: fill the chain-phase PE idle so the HAM
            # clock gate stays open into the next step's matmul burst
            junk = ps_g.tile([U, CW], F32, tag="psx", bufs=1)
            for j in range(4):
                nc.tensor.matmul(junk[:], R[:, rs if j % 2 else zs],
                                 h[0][:], start=(j == 0), stop=(j == 3))
            gru_chains(ps)
            psl, SP = dense_head(t)

    nc.compile()
    return nc


def _host_prep(inputs, gru_kernel, gru_rec_kernel, gru_bias, dv_loc, dv_rho,
               dv_eps, samp_eps):
    """Host-side input preprocessing -> per-core input maps."""
    inputs = np.asarray(inputs, np.float32)
    B = inputs.shape[0]
    assert B == B_FULL, f"kernel compiled for B={B_FULL}, got {B}"
    xT = _round_rd(inputs[:, :T_ENC, 0].T).astype(np.float32)  # [48, B]
    epsT = np.ascontiguousarray(np.asarray(samp_eps, np.float32)[:, :, 0])  # [27, B]

    gru_bias = np.asarray(gru_bias, np.float32)
    b0, b1 = gru_bias[0], gru_bias[1]
    gk = np.asarray(gru_kernel, np.float32)[0]                 # [3U]
    rk = np.asarray(gru_rec_kernel, np.float32)                # [U, 3U]

    # gate reorder {z,r,h} -> {r, -z, h}; z columns negated
    def reorder(m, axis):
        z, r, hh_ = np.split(m, 3, axis=axis)
        return np.concatenate([r, -z, hh_], axis=axis)

    r_w = reorder(rk, 1)                                       # [U, 3U]
    k_row = reorder(gk[None, :], 1)                            # [1, 3U]
    bias_rz = reorder((b0 + b1)[None, :], 1)                   # [1, 3U]
    bias_rz[0, 2 * U :] = 0.0                                  # h-gate bias via ACT
    k2 = np.concatenate([k_row, bias_rz], axis=0)              # [2, 3U]
    # decoder assumes zero r/z input biases (true for this problem);
    # nonzero ones would need the ones-row path in the decoder too.
    assert not np.any(bias_rz[0, : 2 * U]), "nonzero gate biases unsupported"
    k33 = np.zeros((PH, 3 * U), np.float32)
    k33[0] = k_row[0]
    k33[32] = k_row[0]

    gb = np.zeros((U, 2), np.float32)
    gb[:, 0] = b0[2 * U : 3 * U]                               # tanh bias
    gb[:, 1] = b1[2 * U : 3 * U]                               # recurrent h bias

    dv_loc = np.asarray(dv_loc, np.float32)
    dv_rho = np.asarray(dv_rho, np.float32)
    dv_eps = np.asarray(dv_eps, np.float32)
    scale_q = np.float32(1e-5) + np.float32(Q_SCALE) * np.logaddexp(
        np.float32(C_SP) + dv_rho, np.float32(0.0), dtype=np.float32
    )
    w_all = dv_loc[None, :] + scale_q[None, :] * dv_eps        # [28, 258]
    wk = np.ascontiguousarray(
        w_all[:, : 2 * U].reshape(GAMMA, U, 2).transpose(1, 0, 2).reshape(U, 2 * GAMMA)
    )
    # [U, (2t+ch)*33] view: col 0 of each 33-block = the weight, rest 0
    wk33 = np.zeros((U, 2 * GAMMA, PH), np.float32)
    wk33[:, :, 0] = wk
    wk33 = np.ascontiguousarray(wk33.reshape(U, 2 * PH * GAMMA))
    wb33 = np.broadcast_to(w_all[:, 2 * U][None, :], (PH, GAMMA)).copy()
    cvb33 = np.broadcast_to(
        (np.float32(C_SP) + w_all[:, 2 * U + 1])[None, :], (PH, GAMMA)
    ).copy()

    np_rd = np.float16 if _MM_MODE == "fp16" else np.float32
    x2_seq = np.empty((T_ENC, 2, B_FULL), np_rd)
    x2_seq[:, 0, :] = xT
    x2_seq[:, 1, :] = 1.0

    shared = {
        "r_w": _round_rd(r_w),
        "k2_w": _round_rd(k2),
        "k33_w": _round_rd(k33),
        "wk": _round_rd(wk),
        "wk33": _round_rd(wk33),
        "wb33": wb33.astype(np.float32),
        "cvb33": cvb33.astype(np.float32),
        "gb": gb,
        "k_col": np.ascontiguousarray(gk[2 * U :, None]),      # [U,1] K_h col
        "h0_z": np.zeros((U, BC), np_rd),
    }
    in_maps = []
    for c in range(N_CORES):
        sl = slice(c * BC, (c + 1) * BC)
        in_maps.append(
            dict(
                shared,
                x2_seq=np.ascontiguousarray(x2_seq[:, :, sl]),
                xb_seq=np.ascontiguousarray(xT[:, sl].astype(np_rd)),
                eps_seq=np.ascontiguousarray(
                    epsT[:, sl].reshape(GAMMA - 1, NCH, CW)),
            )
        )
    return in_maps, bool(np.any(gb[:, 1] != 0.0)), epsT


def _get_nc(with_b1h=False):
    key = ("nc", with_b1h)
    if key not in _CACHE:
        _CACHE[key] = _build_program(with_b1h)
    return _CACHE[key]


def _postprocess(res_list, epsT):
    """Assemble [B, GAMMA, 2] from per-core {out_E, out_y, out_l27}."""
    out = np.empty((B_FULL, GAMMA, 2), np.float32)
    for c in range(N_CORES):
        sl = slice(c * BC, (c + 1) * BC)
        E = np.asarray(res_list[c]["out_E"], np.float64).reshape(GAMMA, BC)
        y = np.asarray(res_list[c]["out_y"], np.float64).reshape(GAMMA - 1, BC)
        l27 = np.asarray(res_list[c]["out_l27"], np.float64).reshape(BC)
        scale = 1e-5 + OP_SCALE * np.log1p(E)                  # [28, BC]
        loc = np.empty((GAMMA, BC))
        loc[:-1] = y - scale[:-1] * epsT[:, sl]
        loc[-1] = l27
        out[sl, :, 0] = loc.T
        out[sl, :, 1] = scale.T
    return out


def run(inputs_dict, trace=False, trace_kwargs=None):
    in_maps, with_b1h, epsT = _host_prep(**inputs_dict)
    nc = _get_nc(with_b1h)
    res = run_bass_kernel_spmd(
        nc, in_maps, list(range(N_CORES)), trace=trace,
        **(trace_kwargs or {}),
    )
    _CACHE["last_results"] = res
    return _postprocess(res.results, epsT)


def kernel(**inputs):
    return run(inputs, trace=bool(os.environ.get("KERNEL_TRACE")))


# revision 59
# speedup vs baseline: 1.0078x; 1.0037x over previous
"""Trainium2 Bass kernel for nn_FIB_RNN (GRU encoder + autoregressive
sampling decoder with DenseVariational head).

Contract: kernel(**inputs) takes the FULL unsharded inputs (numpy arrays,
keys as in reference.setup_inputs()) and returns the FULL output
[B, GAMMA, 2] float32.

Strategy: pure data parallelism over the batch dim across 8 NeuronCores
(1024 batch rows per core), feature-major GRU state [U=128, batch],
2 batch chunks of 512 pipelined against each other.  The per-chunk
recurrent chain (matmul -> sigmoid -> mul/add -> tanh -> h-update) is
the binding constraint, so the design optimizes chain latency
(873us baseline -> ~685us):

 - fp16 matmul operands (2x stream rate + FWL fast weight loads; the
   10-bit mantissa keeps the recurrent rounding error ~8x below bf16,
   final rel err 1.5e-3 vs the 2e-2 gate).
 - gates reordered {r, -z, h} with z negated: r and u1=1-z come from
   plain sigmoid; the r-half activation is emitted separately so the
   reset-mul can start before the z-half is done.
 - gate input biases ride a ones-row in an augmented K=2 encoder input
   matmul; h-gate bias via the tanh ACT bias (all biases are zero for
   this problem's inputs anyway).
 - decoder head/sample pipeline runs on [33, 512] tiles with chunk 1's
   row at partition 32 via matmul tile_position, so Exp, Ln(x+1),
   AFFINE_MUL_REDUCE and the y-assembly scalar_tensor_tensor all run at
   512-element latency instead of 1024 on a single lane.  (Non-matmul
   engine ops must still start at partition 0 - base-32 ACT/DVE APs
   break on hardware - so those ops cover the whole [33, 512] tile.)
 - softplus via Exp then Ln(x+1), pinned to the
   natural_log_exp_and_others table set (one table-load pair per
   decoder step, loads overlapped with the head matmuls).
 - outputs: raw exp row E and the fp16 feedback sample y are DMA'd;
   the host computes scale = 1e-5+0.05*log1p(E) and reconstructs
   loc = y - scale*eps exactly.
 - chain emission is phase-ordered across chunks (all sigmoids, then
   tt/uu, then tanh, then h-updates) so the in-order ACT/DVE queues
   never head-of-line block a ready op behind the other chunk's stall.
 - dependency-free warming matmuls in BOTH loops fill PE idle gaps to
   keep the HAM activity window busy (cold 1.2 GHz vs warm 2.4 GHz PE;
   measured ~600ns vs ~370ns per [128,128]@[128,512] matmul).  Removing
   the decoder's warming costs >100us - the warmth carries into the
   next step's matmul burst.
 - PSUM plan (8 banks): psrz [128,1024] bufs=2 (4) + psh [128,512]
   bufs=2 (2) + psx/junk [128,512] bufs=1 (1) + phead [33,512] bufs=1
   (1).
"""

import os
import sys
from contextlib import ExitStack

import numpy as np

for _p in ("/opt/trn_rl_repo", "/root/.axon_site/_ro/trn_rl_repo"):
    if os.path.isdir(_p) and _p not in sys.path:
        sys.path.insert(0, _p)

import concourse.bass as bass
import concourse.tile as tile
from concourse import bacc, mybir
from concourse.bass_utils import run_bass_kernel_spmd

F32 = mybir.dt.float32
AF = mybir.ActivationFunctionType
ALU = mybir.AluOpType
_MM_MODE = os.environ.get("KERNEL_MM_DT", "fp16")
RD = {"fp16": mybir.dt.float16, "f32r": mybir.dt.float32r}[_MM_MODE]
RD16 = mybir.dt.float16 if _MM_MODE == "fp16" else F32

U = 128                    # rnn units
T_ENC = 48                 # encoder steps
GAMMA = 28                 # decoder outputs (27 sampled feedback steps)
N_CORES = 8
B_FULL = 8192
BC = B_FULL // N_CORES     # 1024 batch rows per core
CW = 512                   # chunk width (PSUM bank = 512 fp32)
NCH = BC // CW             # 2 chunks per core
PH = 33                    # head tile partition span ({0, 32} rows used)
C_SP = float(np.log(np.expm1(1.0)))  # softplus^-1(1.0)
Q_SCALE = 0.02
OP_SCALE = 0.05

_CACHE = {}


def _round_rd(a):
    """Cast fp32 array to the matmul operand dtype's numpy storage."""
    a = np.ascontiguousarray(a, np.float32)
    if _MM_MODE == "fp16":
        return np.ascontiguousarray(a.astype(np.float16))
    bits = a.view(np.uint32)
    out = ((bits.astype(np.uint64) + 0x800) & 0xFFFFF000).astype(np.uint32)
    return out.view(np.float32)


def _pin_act_tables(arch):
    """Hide Exp/Ln from the single-function table sets so the compiler's
    table-load placement resolves both to natural_log_exp_and_others
    (one load covers the decoder's Exp+Ln pair).  Mutates the cached
    dict in place; set positions (= walrus set ids) are unchanged, and
    the real on-device tables still contain the hidden entries, so this
    only steers placement, never correctness."""
    from concourse.hw_specs import get_activation_tables

    tabs = get_activation_tables(arch)
    for name in ("exp_and_others", "exp_and_friends"):
        if name in tabs:
            tabs[name].discard(AF.Exp)
    if "natural_log" in tabs:
        tabs["natural_log"].discard(AF.Ln)


def _build_program(with_b1h):
    """Build + schedule the single-core Bass program (shared by all 8
    cores; per-core data differs only through the input tensors)."""
    nc = bacc.Bacc("TRN2", target_bir_lowering=False, debug=False)
    _pin_act_tables(nc.m.arch)

    # DRAM tensors.  Gate order everywhere is {r, -z, h}.
    x2_seq = nc.dram_tensor("x2_seq", [T_ENC, 2, BC], RD, kind="ExternalInput").ap()
    xb_seq = nc.dram_tensor("xb_seq", [T_ENC, BC], RD, kind="ExternalInput").ap()
    eps_seq = nc.dram_tensor("eps_seq", [GAMMA - 1, NCH, CW], F32, kind="ExternalInput").ap()
    r_w = nc.dram_tensor("r_w", [U, 3 * U], RD, kind="ExternalInput").ap()
    k2_w = nc.dram_tensor("k2_w", [2, 3 * U], RD, kind="ExternalInput").ap()
    k33_w = nc.dram_tensor("k33_w", [PH, 3 * U], RD, kind="ExternalInput").ap()
    wk = nc.dram_tensor("wk", [U, 2 * GAMMA], RD, kind="ExternalInput").ap()
    wk33 = nc.dram_tensor("wk33", [U, 2 * PH * GAMMA], RD, kind="ExternalInput").ap()
    wb33 = nc.dram_tensor("wb33", [PH, GAMMA], F32, kind="ExternalInput").ap()
    cvb33 = nc.dram_tensor("cvb33", [PH, GAMMA], F32, kind="ExternalInput").ap()
    gb = nc.dram_tensor("gb", [U, 2], F32, kind="ExternalInput").ap()
    k_col = nc.dram_tensor("k_col", [U, 1], F32, kind="ExternalInput").ap()
    h0_z = nc.dram_tensor("h0_z", [U, BC], RD, kind="ExternalInput").ap()
    out_E = nc.dram_tensor("out_E", [GAMMA, NCH, CW], F32, kind="ExternalOutput").ap()
    out_y = nc.dram_tensor("out_y", [GAMMA - 1, NCH, CW], RD, kind="ExternalOutput").ap()
    out_l27 = nc.dram_tensor("out_l27", [NCH, CW], F32, kind="ExternalOutput").ap()

    with tile.TileContext(nc) as tc, ExitStack() as es:
        consts = es.enter_context(tc.tile_pool(name="consts", bufs=1))
        R = consts.tile([U, 3 * U], RD)
        K2 = consts.tile([2, 3 * U], RD)
        K33 = consts.tile([PH, 3 * U], RD)
        WK = consts.tile([U, 2 * GAMMA], RD)
        WK33 = consts.tile([U, 2 * PH * GAMMA], RD)
        WB = consts.tile([PH, GAMMA], F32)
        CVB = consts.tile([PH, GAMMA], F32)
        GB = consts.tile([U, 2], F32)
        KC = consts.tile([U, 1], F32)
        EP = consts.tile([PH, CW], F32, name="ep")
        ACC = consts.tile([PH, 1], F32)
        ACC128 = consts.tile([U, 1], F32)
        nc.sync.dma_start(R[:], r_w[:])
        nc.sync.dma_start(K2[:], k2_w[:])
        nc.sync.dma_start(K33[:], k33_w[:])
        nc.sync.dma_start(WK[:], wk[:])
        nc.sync.dma_start(WK33[:], wk33[:])
        nc.sync.dma_start(WB[:], wb33[:])
        nc.sync.dma_start(CVB[:], cvb33[:])
        nc.sync.dma_start(GB[:], gb[:])
        nc.sync.dma_start(KC[:], k_col[:])
        nc.vector.memset(EP[:], 0.0)

        hpool = es.enter_context(tc.tile_pool(name="h", bufs=3))
        gates = es.enter_context(tc.tile_pool(name="gates", bufs=2))
        samp = es.enter_context(tc.tile_pool(name="samp", bufs=2))
        stage = es.enter_context(tc.tile_pool(name="stage", bufs=3))
        ps_g = es.enter_context(tc.tile_pool(name="psg", bufs=2, space="PSUM"))

        h = []
        for c in range(NCH):
            hc = hpool.tile([U, CW], RD, tag=f"h{c}")
            nc.sync.dma_start(hc[:], h0_z[:, bass.ts(c, CW)])
            h.append(hc)

        rs = bass.ts(0, U)       # gate column ranges in R/K: {r, -z, h}
        zs = bass.ts(1, U)
        hs = bass.ts(2, U)

        def gru_mm(c, x2=None, Y=None):
            """Gate matmuls for chunk c.  Encoder: x2 [2,BC] (row1=ones
            for biases); h-gate x-term comes via the xb STT instead of a
            matmul.  Decoder: Y [33,CW] fp16 (row 32c = y_c).
            Returns (psrz, psh, psx)."""
            hc = h[c]
            psrz = ps_g.tile([U, 2 * CW], F32, tag="psrz", bufs=2)
            psh = ps_g.tile([U, CW], F32, tag="psh", bufs=2)
            psx = None
            if Y is None:
                cs = bass.ts(c, CW)
                nc.tensor.matmul(psrz[:, 0:CW], K2[:, rs], x2[:, cs],
                                 start=True, stop=False)
                nc.tensor.matmul(psrz[:, 0:CW], R[:, rs], hc[:],
                                 start=False, stop=True)
                nc.tensor.matmul(psrz[:, CW:], K2[:, zs], x2[:, cs],
                                 start=True, stop=False)
                nc.tensor.matmul(psrz[:, CW:], R[:, zs], hc[:],
                                 start=False, stop=True)
                nc.tensor.matmul(psh[:], R[:, hs], hc[:], start=True, stop=True)
            else:
                p = 32 * c
                yr = Y[p : p + 1, :]
                kr = K33[p : p + 1, :]
                tp = (p, 0)
                psx = ps_g.tile([U, CW], F32, tag="psx", bufs=1)
                # R@h first (ready early), then K@y (y arrives late)
                nc.tensor.matmul(psrz[:, 0:CW], R[:, rs], hc[:],
                                 start=True, stop=False)
                nc.tensor.matmul(psrz[:, CW:], R[:, zs], hc[:],
                                 start=True, stop=False)
                nc.tensor.matmul(psh[:], R[:, hs], hc[:], start=True, stop=True)
                nc.tensor.matmul(psrz[:, 0:CW], kr[:, rs], yr,
                                 start=False, stop=True, tile_position=tp)
                nc.tensor.matmul(psrz[:, CW:], kr[:, zs], yr,
                                 start=False, stop=True, tile_position=tp)
                nc.tensor.matmul(psx[:], kr[:, hs], yr,
                                 start=True, stop=True, tile_position=tp)
            return psrz, psh, psx

        def gru_sig(c, psrz):
            """Gate sigmoids for chunk c (emitted for both chunks before
            any tanh so the in-order ACT queue never head-of-line blocks
            a ready sigmoid behind the other chunk's tanh)."""
            G = gates.tile([U, 2 * CW], RD16, tag=f"G_{c}")
            # r-half first: the reset-mul only needs this half
            nc.scalar.activation(G[:, 0:CW], psrz[:, 0:CW], AF.Sigmoid,
                                 bias=0.0, scale=1.0)
            nc.scalar.activation(G[:, CW:], psrz[:, CW:], AF.Sigmoid,
                                 bias=0.0, scale=1.0)
            return G

        def gru_mid(c, G, psh, psx, xb=None):
            """tt/uu for chunk c (both chunks emitted before any tanh)."""
            hrec = psh
            if with_b1h:
                hb = gates.tile([U, CW], F32, tag=f"hb_{c}")
                nc.vector.tensor_scalar(hb[:], psh[:], GB[:, 1:2], None,
                                        op0=ALU.add)
                hrec = hb
            tt = gates.tile([U, CW], F32, tag=f"t_{c}")
            nc.vector.tensor_mul(tt[:], G[:, 0:CW], hrec[:])
            uu = gates.tile([U, CW], F32, tag=f"u_{c}")
            if psx is None:
                nc.vector.scalar_tensor_tensor(
                    uu[:], xb[:, bass.ts(c, CW)], KC[:, 0:1], tt[:],
                    op0=ALU.mult, op1=ALU.add,
                )
            else:
                nc.vector.tensor_add(uu[:], tt[:], psx[:])
            # b = (1-u1)*h depends only on the z-sigmoid and h, so it
            # runs here, off the tanh-side critical path
            b = gates.tile([U, CW], RD16, tag=f"b_{c}")
            nc.vector.affine_mul_reduce(
                b[:], ACC128[:], G[:, CW:], h[c][:], -1.0, 1.0,
            )
            return uu, b

        def gru_tanh(c, uu):
            hh = gates.tile([U, CW], RD16, tag=f"hh_{c}")
            nc.scalar.activation(hh[:], uu[:], AF.Tanh, bias=GB[:, 0:1],
                                 scale=1.0)
            return hh

        def gru_upd(c, G, hh, b):
            # h2 = u1*hh + (1-u1)*h: only two ops follow the tanh
            a = gates.tile([U, CW], RD16, tag=f"d_{c}")
            nc.vector.tensor_mul(a[:], G[:, CW:], hh[:])
            h2 = hpool.tile([U, CW], RD, tag=f"h{c}")
            nc.vector.tensor_add(h2[:], a[:], b[:])
            h[c] = h2

        def gru_chains(ps, xb=None):
            """Phase-ordered chains for both chunks."""
            G = [gru_sig(c, ps[c][0]) for c in range(NCH)]
            ub = [gru_mid(c, G[c], ps[c][1], ps[c][2], xb=xb)
                  for c in range(NCH)]
            hh = [gru_tanh(c, ub[c][0]) for c in range(NCH)]
            for c in range(NCH):
                gru_upd(c, G[c], hh[c], ub[c][1])

        def dense_head(t):
            """DenseVariational head for step t on [33, CW] tiles
            (chunk c at partition 32c).  Returns (psl, SP)."""
            # chunk0's matmul is M=33 with a zero-padded lhsT so every
            # PSUM row is written (CoreSim rejects partial-init reads);
            # chunk1 overwrites row 32.  Same 512-col stream cost.
            psv = ps_g.tile([PH, CW], F32, tag="phead", bufs=1)
            v33 = bass.ts(2 * t + 1, PH)
            nc.tensor.matmul(psv[:], WK33[:, v33], h[0][:],
                             start=True, stop=True)
            nc.tensor.matmul(psv[32:33, :], WK[:, 2 * t + 1 : 2 * t + 2],
                             h[1][:], start=True, stop=True,
                             tile_position=(0, 32))
            E = samp.tile([PH, CW], F32, tag="E")
            nc.scalar.activation(E[:], psv[:], AF.Exp,
                                 bias=CVB[:, t : t + 1], scale=1.0)
            for c in range(NCH):
                nc.sync.dma_start(out_E[t, c, :],
                                  E[32 * c : 32 * c + 1, :])
            psl = ps_g.tile([PH, CW], F32, tag="phead", bufs=1)
            l33 = bass.ts(2 * t, PH)
            nc.tensor.matmul(psl[:], WK33[:, l33], h[0][:],
                             start=True, stop=True)
            nc.tensor.matmul(psl[32:33, :], WK[:, 2 * t : 2 * t + 1],
                             h[1][:], start=True, stop=True,
                             tile_position=(0, 32))
            if t == GAMMA - 1:
                l27 = samp.tile([PH, CW], F32, tag="l27")
                nc.vector.tensor_scalar(l27[:], psl[:], WB[:, t : t + 1],
                                        None, op0=ALU.add)
                for c in range(NCH):
                    nc.sync.dma_start(out_l27[c, :],
                                      l27[32 * c : 32 * c + 1, :])
                return None, None
            SP = samp.tile([PH, CW], F32, tag="SP")
            nc.scalar.activation(SP[:], E[:], AF.Ln, bias=1.0, scale=1.0)
            return psl, SP

        def sample(t, psl, SP):
            """Y rows {0,32} = (psl + wb0_t) + (0.05*sp + 1e-5)*eps_t."""
            for c in range(NCH):
                nc.sync.dma_start(EP[32 * c : 32 * c + 1, :],
                                  eps_seq[t, c, :])
            M = samp.tile([PH, CW], F32, tag="M")
            nc.vector.affine_mul_reduce(
                M[:], ACC[:], SP[:], EP[:], OP_SCALE, 1e-5,
            )
            Y = samp.tile([PH, CW], RD, tag="Y")
            nc.vector.scalar_tensor_tensor(
                Y[:], psl[:], WB[:, t : t + 1], M[:],
                op0=ALU.add, op1=ALU.add,
            )
            for c in range(NCH):
                nc.sync.dma_start(out_y[t, c, :], Y[32 * c : 32 * c + 1, :])
            return Y

        # ---- encoder: 48 GRU steps over the input sequence ----
        for t in range(T_ENC):
            x2 = stage.tile([2, BC], RD, tag="x2")
            nc.sync.dma_start(x2[:], x2_seq[t, :, :])
            xb = stage.tile([U, BC], RD, tag="xb")
            nc.sync.dma_start(xb[:], xb_seq[t : t + 1, :].partition_broadcast(U))
            ps = [gru_mm(c, x2=x2) for c in range(NCH)]
            # dependency-free warming matmuls into a spare bank: they run
            # while the PE would otherwise stall on h2, keeping the HAM
            # activity window busy so real matmuls stream at 2.4 GHz.
            junk = ps_g.tile([U, CW], F32, tag="psx", bufs=1)
            for j in range(4):
                nc.tensor.matmul(junk[:], R[:, rs if j % 2 else zs],
                                 xb[:, 0:CW], start=(j == 0),
                                 stop=(j == 3))
            gru_chains(ps, xb=xb)

        # ---- decoder: dense head + 27 sampled feedback GRU steps ----
        psl, SP = dense_head(0)
        for t in range(1, GAMMA):
            Y = sample(t - 1, psl, SP)
            ps = [gru_mm(c, Y=Y) for c in range(NCH)]
            # warming matmuls: fill the chain-phase PE idle so the HAM
            # clock gate stays open into the next step's matmul burst
            junk = ps_g.tile([U, CW], F32, tag="psx", bufs=1)
            for j in range(4):
                nc.tensor.matmul(junk[:], R[:, rs if j % 2 else zs],
                                 h[0][:], start=(j == 0), stop=(j == 3))
            gru_chains(ps)
            psl, SP = dense_head(t)

    nc.compile()
    return nc


def _host_prep(inputs, gru_kernel, gru_rec_kernel, gru_bias, dv_loc, dv_rho,
               dv_eps, samp_eps):
    """Host-side input preprocessing -> per-core input maps."""
    inputs = np.asarray(inputs, np.float32)
    B = inputs.shape[0]
    assert B == B_FULL, f"kernel compiled for B={B_FULL}, got {B}"
    xT = _round_rd(inputs[:, :T_ENC, 0].T).astype(np.float32)  # [48, B]
    epsT = np.ascontiguousarray(np.asarray(samp_eps, np.float32)[:, :, 0])  # [27, B]

    gru_bias = np.asarray(gru_bias, np.float32)
    b0, b1 = gru_bias[0], gru_bias[1]
    gk = np.asarray(gru_kernel, np.float32)[0]                 # [3U]
    rk = np.asarray(gru_rec_kernel, np.float32)                # [U, 3U]

    # gate reorder {z,r,h} -> {r, -z, h}; z columns negated
    def reorder(m, axis):
        z, r, hh_ = np.split(m, 3, axis=axis)
        return np.concatenate([r, -z, hh_], axis=axis)

    r_w = reorder(rk, 1)                                       # [U, 3U]
    k_row = reorder(gk[None, :], 1)                            # [1, 3U]
    bias_rz = reorder((b0 + b1)[None, :], 1)                   # [1, 3U]
    bias_rz[0, 2 * U :] = 0.0                                  # h-gate bias via ACT
    k2 = np.concatenate([k_row, bias_rz], axis=0)              # [2, 3U]
    # decoder assumes zero r/z input biases (true for this problem);
    # nonzero ones would need the ones-row path in the decoder too.
    assert not np.any(bias_rz[0, : 2 * U]), "nonzero gate biases unsupported"
    k33 = np.zeros((PH, 3 * U), np.float32)
    k33[0] = k_row[0]
    k33[32] = k_row[0]

    gb = np.zeros((U, 2), np.float32)
    gb[:, 0] = b0[2 * U : 3 * U]                               # tanh bias
    gb[:, 1] = b1[2 * U : 3 * U]                               # recurrent h bias

    dv_loc = np.asarray(dv_loc, np.float32)
    dv_rho = np.asarray(dv_rho, np.float32)
    dv_eps = np.asarray(dv_eps, np.float32)
    scale_q = np.float32(1e-5) + np.float32(Q_SCALE) * np.logaddexp(
        np.float32(C_SP) + dv_rho, np.float32(0.0), dtype=np.float32
    )
    w_all = dv_loc[None, :] + scale_q[None, :] * dv_eps        # [28, 258]
    wk = np.ascontiguousarray(
        w_all[:, : 2 * U].reshape(GAMMA, U, 2).transpose(1, 0, 2).reshape(U, 2 * GAMMA)
    )
    # [U, (2t+ch)*33] view: col 0 of each 33-block = the weight, rest 0
    wk33 = np.zeros((U, 2 * GAMMA, PH), np.float32)
    wk33[:, :, 0] = wk
    wk33 = np.ascontiguousarray(wk33.reshape(U, 2 * PH * GAMMA))
    wb33 = np.broadcast_to(w_all[:, 2 * U][None, :], (PH, GAMMA)).copy()
    cvb33 = np.broadcast_to(
        (np.float32(C_SP) + w_all[:, 2 * U + 1])[None, :], (PH, GAMMA)
    ).copy()

    np_rd = np.float16 if _MM_MODE == "fp16" else np.float32
    x2_seq = np.empty((T_ENC, 2, B_FULL), np_rd)
    x2_seq[:, 0, :] = xT
    x2_seq[:, 1, :] = 1.0

    shared = {
        "r_w": _round_rd(r_w),
        "k2_w": _round_rd(k2),
        "k33_w": _round_rd(k33),
        "wk": _round_rd(wk),
        "wk33": _round_rd(wk33),
        "wb33": wb33.astype(np.float32),
        "cvb33": cvb33.astype(np.float32),
        "gb": gb,
        "k_col": np.ascontiguousarray(gk[2 * U :, None]),      # [U,1] K_h col
        "h0_z": np.zeros((U, BC), np_rd),
    }
    in_maps = []
    for c in range(N_CORES):
        sl = slice(c * BC, (c + 1) * BC)
        in_maps.append(
            dict(
                shared,
                x2_seq=np.ascontiguousarray(x2_seq[:, :, sl]),
                xb_seq=np.ascontiguousarray(xT[:, sl].astype(np_rd)),
                eps_seq=np.ascontiguousarray(
                    epsT[:, sl].reshape(GAMMA - 1, NCH, CW)),
            )
        )
    return in_maps, bool(np.any(gb[:, 1] != 0.0)), epsT


def _get_nc(with_b1h=False):
    key = ("nc", with_b1h)
    if key not in _CACHE:
        _CACHE[key] = _build_program(with_b1h)
    return _CACHE[key]


def _postprocess(res_list, epsT):
    """Assemble [B, GAMMA, 2] from per-core {out_E, out_y, out_l27}."""
    out = np.empty((B_FULL, GAMMA, 2), np.float32)
    for c in range(N_CORES):
        sl = slice(c * BC, (c + 1) * BC)
        E = np.asarray(res_list[c]["out_E"], np.float64).reshape(GAMMA, BC)
        y = np.asarray(res_list[c]["out_y"], np.float64).reshape(GAMMA - 1, BC)
        l27 = np.asarray(res_list[c]["out_l27"], np.float64).reshape(BC)
        scale = 1e-5 + OP_SCALE * np.log1p(E)                  # [28, BC]
        loc = np.empty((GAMMA, BC))
        loc[:-1] = y - scale[:-1] * epsT[:, sl]
        loc[-1] = l27
        out[sl, :, 0] = loc.T
        out[sl, :, 1] = scale.T
    return out


def run(inputs_dict, trace=False, trace_kwargs=None):
    in_maps, with_b1h, epsT = _host_prep(**inputs_dict)
    nc = _get_nc(with_b1h)
    res = run_bass_kernel_spmd(
        nc, in_maps, list(range(N_CORES)), trace=trace,
        **(trace_kwargs or {}),
    )
    _CACHE["last_results"] = res
    return _postprocess(res.results, epsT)


def kernel(**inputs):
    return run(inputs, trace=bool(os.environ.get("KERNEL_TRACE")))
